# revision 1
# baseline (speedup 1.0000x reference)
"""Trainium2 Bass kernel for nn_Attention_1494648619518.

Fake-quantized (4-bit) multi-head attention:
    qkv = fq(x) @ fq(ww1).T ; per-head softmax(fq(q) fq(k)^T / 8) ;
    out = fq(a) @ fq(v) ; y = fq(out) @ fq(ww2).T + bb2

Implementation notes:
  * fq(t, s, z) = s * clamp(round(t/s), -round(z), 15-round(z)).  Every
    fake-quantized tensor is (scale) x (small integer in [-8..15]).  The
    integers are exact in bf16, so all matmuls run in bf16 on integer
    values with fp32 PSUM accumulation -- bit-exact integer dot products
    (all magnitudes stay far below 2^24).
  * round-to-nearest-even to integer is one fused op: add 192.0 and cast
    fp32->bf16 (bf16 spacing is exactly 1.0 on [128,256)), then subtract
    192 where an unoffset operand is needed.  For the two large tensors
    (quantized softmax `ia`, attention output `ix2`) the +192 offset is
    kept and cancelled after the matmul via column-sum corrections.
  * softmax: dots are exact integers; ACT exp folds the combined scale and
    produces the row sum via accum_out.  No max-subtraction is needed:
    |dots_int| <= 64*8*8 = 4096 bounds the exp argument (~11.5 here).
  * sharding: 8 cores = (4 batches) x (2 sequence halves).  Each core
    computes q for its 1024 rows and k/v for all 2048 rows of its batch
    (duplicate k/v compute instead of a collective), writing a disjoint
    1024x768 output slice.
  * data-dependent skip: ia = round(a/sa) is identically zero unless some
    attention weight exceeds sa/2 (=1/30 here).  sum(ia) is accumulated
    exactly during quantization; if zero, attention-out and the final
    projection are exactly zero and the output is bb2 broadcast, so a
    runtime branch skips the dense path.  Otherwise the dense path runs
    (recomputing attention tiles per head, then transposed matmuls).
"""

import math
import os
import sys
from contextlib import ExitStack

import numpy as np

for _p in ("/opt/trn_rl_repo", "/root/.axon_site/_ro/trn_rl_repo"):
    if os.path.isdir(_p) and _p not in sys.path:
        sys.path.insert(0, _p)

import concourse.bass as bass
import concourse.tile as tile
from concourse import bacc, bass_isa, bass_utils, mybir

dt = mybir.dt
AF = mybir.ActivationFunctionType
ALU = mybir.AluOpType

P = 128
QMAX = 15.0
OFF = 192.0  # bf16 int-round offset: ints+192 in [128,256) where bf16 spacing == 1
EPS = 0.4995  # clamp shrink so min(t, hi+EPS) always rounds to <= hi


def _f32(x):
    return float(np.float32(x))


class QP:
    """Params for one fake_quant(t, s, z) tensor."""

    def __init__(self, s, z):
        self.s = _f32(s)
        self.inv = float(np.float32(np.float64(1.0) / np.float64(self.s)))
        zr = float(np.round(np.float64(z)))
        self.lo = -zr
        self.hi = QMAX - zr


class Cfg:
    def __init__(self, dim, heads, dh, n, nl, scales, conditional, num_devices):
        self.dim = dim
        self.heads = heads
        self.dh = dh
        self.inner = heads * dh
        self.j1 = 3 * self.inner
        self.n = n      # k/v rows per core (full batch sequence)
        self.nl = nl    # q rows per core (local output rows)
        self.scales = dict(scales)
        self.conditional = conditional
        self.num_devices = num_devices
        assert dh == 64
        assert dim % P == 0 and self.inner % P == 0
        assert n % P == 0 and nl % P == 0
        assert heads % 2 == 0

    def key(self):
        return (
            self.dim, self.heads, self.dh, self.n, self.nl, self.conditional,
            self.num_devices,
            tuple(sorted((k, _f32(v)) for k, v in self.scales.items())),
        )


def _chunks(total, maxc):
    out = []
    o = 0
    while o < total:
        c = min(maxc, total - o)
        out.append((o, c))
        o += c
    return out


def build(cfg: Cfg):
    sc = cfg.scales
    qx1 = QP(sc["sx1"], sc["zx1"])
    qw1 = QP(sc["sw1"], sc["zw1"])
    qq = QP(sc["sq"], sc["zq"])
    qk = QP(sc["sk"], sc["zk"])
    qa = QP(sc["sa"], sc["za"])
    qv = QP(sc["sv"], sc["zv"])
    qx2 = QP(sc["sx2"], sc["zx2"])
    qw2 = QP(sc["sw2"], sc["zw2"])

    # The fused (no-clamp) a-quantization requires that the clamp can never
    # bind: a in [0,1] so a/sa <= 1/sa <= hi+0.4999 and lo <= 0.  Holds for
    # the calibrated sa=1/15, za=0 of this model.
    assert qa.lo <= 0.0 and (1.0 / qa.s) <= qa.hi + 0.4999, (
        "general clamping a-quantization not implemented for these scales")

    dim, inner, heads, dh = cfg.dim, cfg.inner, cfg.heads, cfg.dh
    n, nl, j1 = cfg.n, cfg.nl, cfg.j1
    n_i = dim // P
    n_jq = inner // P
    n_nt = nl // P
    n_mc = n // P
    n_tiles = heads * n_nt

    f64 = np.float64
    s_q = float(np.float32(f64(qx1.s) * f64(qw1.s) / f64(qq.s)))
    s_k = float(np.float32(f64(qx1.s) * f64(qw1.s) / f64(qk.s)))
    s_v = float(np.float32(f64(qx1.s) * f64(qw1.s) / f64(qv.s)))
    alpha = float(np.float32(f64(qq.s) * f64(qk.s) / f64(math.sqrt(dh))))
    inv_sa = float(np.float32(f64(1.0) / f64(qa.s)))
    beta = float(np.float32(f64(qa.s) * f64(qv.s) / f64(qx2.s)))
    gamma = float(np.float32(f64(qx2.s) * f64(qw2.s)))

    nc = bacc.Bacc(
        "TRN2", target_bir_lowering=False, debug=False,
        enable_asserts=True, num_devices=cfg.num_devices)
    xb = nc.dram_tensor("xb", [n, dim], dt.float32, kind="ExternalInput").ap()
    xq = nc.dram_tensor("xq", [nl, dim], dt.float32, kind="ExternalInput").ap()
    ww1 = nc.dram_tensor("ww1", [j1, dim], dt.float32, kind="ExternalInput").ap()
    ww2 = nc.dram_tensor("ww2", [dim, inner], dt.float32, kind="ExternalInput").ap()
    bb2 = nc.dram_tensor("bb2", [1, dim], dt.float32, kind="ExternalInput").ap()
    out_d = nc.dram_tensor("out", [nl, dim], dt.float32, kind="ExternalOutput").ap()
    flag_d = nc.dram_tensor("flagdbg", [1, 1], dt.float32, kind="ExternalOutput").ap()

    with tile.TileContext(nc) as tc, ExitStack() as ctx:
        persist = ctx.enter_context(tc.tile_pool(name="persist", bufs=1))
        consts = ctx.enter_context(tc.tile_pool(name="consts", bufs=1))
        spool = ctx.enter_context(tc.tile_pool(name="spool", bufs=8))
        epool = ctx.enter_context(tc.tile_pool(name="epool", bufs=2))
        iapool = ctx.enter_context(tc.tile_pool(name="iapool", bufs=2))

        # -------- persistent quantized tensors (bf16 integer values) ------
        iqT = [persist.tile([P, nl], dt.bfloat16, tag=f"iqT{j}", name=f"iqT{j}") for j in range(n_jq)]
        ikT = [persist.tile([P, n], dt.bfloat16, tag=f"ikT{j}", name=f"ikT{j}") for j in range(n_jq)]
        iw2T = [persist.tile([P, dim], dt.bfloat16, tag=f"iw2T{j}", name=f"iw2T{j}") for j in range(n_jq)]
        iv_i = [persist.tile([P, n_mc, P], dt.bfloat16, tag=f"ivi{j}", name=f"ivi{j}") for j in range(n_jq)]
        fsums = persist.tile([P, n_tiles], dt.float32, tag="fsums")
        pcorr = [persist.tile([P, 1], dt.float32, tag=f"pcorr{j}", name=f"pcorr{j}") for j in range(n_jq)]

        ones_b = consts.tile([P, 1], dt.bfloat16)
        nc.vector.memset(ones_b[:], 1.0)
        ones_row = consts.tile([1, P], dt.float32)
        nc.vector.memset(ones_row[:], 1.0)
        bb2row = consts.tile([1, dim], dt.float32)
        nc.sync.dma_start(bb2row[:], bb2[:])
        crow = consts.tile([1, dim], dt.float32)
        cbcast = consts.tile([P, dim], dt.float32)
        bcast = consts.tile([P, dim], dt.float32)
        flag_sb = consts.tile([P, 1], dt.float32)
        f3 = consts.tile([P, 1], dt.float32)

        def quantize(dst, src, lo, hi, inv, padj=None, keep_offset=False,
                     tpool=None):
            """dst(bf16) = clamp(round(src*inv [+padj]), lo, hi) [+192].

            src is fp32 (PSUM or SBUF); p1 on DVE, p2/p3 on GpSimd.
            """
            tpool = tpool if tpool is not None else sstage
            pd = src.shape[0]
            fd = dst.free_size()
            t1 = tpool.tile([P, fd], dt.float32, tag="qt1")
            if padj is None:
                nc.vector.tensor_scalar(
                    t1[:pd, :], src, inv, hi + EPS, ALU.mult, ALU.min)
            else:
                nc.vector.tensor_scalar(
                    t1[:pd, :], src, inv, padj, ALU.mult, ALU.add)
                nc.vector.tensor_scalar_min(t1[:pd, :], t1[:pd, :], hi + EPS)
            if keep_offset:
                nc.gpsimd.tensor_scalar(
                    dst, t1[:pd, :], lo - EPS, OFF, ALU.max, ALU.add)
            else:
                t2 = tpool.tile([P, fd], dt.bfloat16, tag="qt2")
                nc.gpsimd.tensor_scalar(
                    t2[:pd, :], t1[:pd, :], lo - EPS, OFF, ALU.max, ALU.add)
                nc.gpsimd.tensor_scalar_add(dst, t2[:pd, :], -OFF)

        def attn_tile(h, nt, ps_pool, accum_idx, need_ia=False):
            jt = h // 2
            po = 64 * (h % 2)
            psd = ps_pool.tile([P, n], dt.float32, tag="dots", name="psd")
            for mo, mw in _chunks(n, 512):
                nc.tensor.matmul(
                    psd[:, mo:mo + mw],
                    iqT[jt][po:po + 64, nt * P:(nt + 1) * P],
                    ikT[jt][po:po + 64, mo:mo + mw],
                    start=True, stop=True)
            e = epool.tile([P, n], dt.float32, tag="e", name="e")
            S = spool.tile([P, 1], dt.float32, tag="S", name="S")
            nc.scalar.activation(e[:], psd[:], AF.Exp, bias=0.0,
                                 scale=alpha, accum_out=S[:])
            r = spool.tile([P, 1], dt.float32, tag="r", name="r")
            nc.vector.reciprocal(r[:], S[:])
            r15 = spool.tile([P, 1], dt.float32, tag="r15", name="r15")
            nc.vector.tensor_scalar_mul(r15[:], r[:], inv_sa)
            ia = iapool.tile([P, n], dt.bfloat16, tag="ia", name="ia")
            if accum_idx is not None and not need_ia:
                # NB: with accum_out, (op1, scalar2) become the reduce op and
                # its initial value -- out is just in*scalar1, and the accum
                # is the per-partition max of bf16(a/sa).
                nc.vector.tensor_scalar(
                    ia[:], e[:], r15[:], 0.0, ALU.mult, ALU.max,
                    accum_out=fsums[:, accum_idx:accum_idx + 1])
                return ia
            nc.vector.tensor_scalar(ia[:], e[:], r15[:], OFF,
                                    ALU.mult, ALU.add)
            if accum_idx is not None:
                junk = iapool.tile([P, n], dt.bfloat16, tag="ia", name="junk")
                nc.vector.tensor_scalar(
                    junk[:], e[:], r15[:], 0.0, ALU.mult, ALU.max,
                    accum_out=fsums[:, accum_idx:accum_idx + 1])
            return ia

        def emit_flag():
            nc.vector.tensor_reduce(
                f3[:], fsums[:], mybir.AxisListType.X, ALU.max)
            nc.gpsimd.partition_all_reduce(
                flag_sb[:], f3[:], channels=P,
                reduce_op=bass_isa.ReduceOp.max)
            nc.sync.dma_start(flag_d[:], flag_sb[0:1, 0:1])

        # ====== phase A: prologue + QKV, attention interleaved per j-tile ==
        with tc.tile_pool(name="proj", bufs=1) as projp, \
                tc.tile_pool(name="pstage", bufs=2) as pstage, \
                tc.tile_pool(name="ps_big", bufs=2, space="PSUM") as ps_big:
            ixT = [projp.tile([P, n], dt.bfloat16, tag=f"ixT{i}", name=f"ixT{i}") for i in range(n_i)]
            ixqT = [projp.tile([P, nl], dt.bfloat16, tag=f"ixqT{i}", name=f"ixqT{i}") for i in range(n_i)]
            iw1T = [projp.tile([P, j1], dt.bfloat16, tag=f"iw1T{i}", name=f"iw1T{i}") for i in range(n_i)]

            def load_quant_transpose(src_dram, rows, cols, qp, put_block):
                # alternate HWDGE issue between SP and ACT sequencers -- the
                # prologue is DMA-issue bound and ACT is idle here
                for rt in range(rows // P):
                    eng = nc.sync if rt % 2 == 0 else nc.scalar
                    xf = pstage.tile([P, cols], dt.float32, tag="ldx", name="xf")
                    eng.dma_start(xf[:], src_dram[rt * P:(rt + 1) * P, :])
                    iq_ = pstage.tile([P, cols], dt.bfloat16, tag="ixq", name="iq_")
                    quantize(iq_[:], xf[:], qp.lo, qp.hi, qp.inv, tpool=pstage)
                    for cc in range(cols // P):
                        put_block(rt, cc, iq_[:, cc * P:(cc + 1) * P])

            def _teng(i):
                return nc.sync if i % 2 == 0 else nc.scalar

            # order by hot-path criticality: the first dots needs all of
            # xb (16 tiles, the longest pole), all of xq, and only the first
            # couple of ww1 row-tiles (subtile deps release j-slices early)
            load_quant_transpose(
                xb, n, dim, qx1,
                lambda rt, ic, blk: _teng(ic).dma_start_transpose(
                    ixT[ic][:, rt * P:(rt + 1) * P], blk))
            load_quant_transpose(
                xq, nl, dim, qx1,
                lambda rt, ic, blk: _teng(ic).dma_start_transpose(
                    ixqT[ic][:, rt * P:(rt + 1) * P], blk))
            load_quant_transpose(
                ww1, j1, dim, qw1,
                lambda rt, ic, blk: _teng(ic).dma_start_transpose(
                    iw1T[ic][:, rt * P:(rt + 1) * P], blk))
            load_quant_transpose(
                ww2, dim, inner, qw2,
                lambda rt, jc, blk: _teng(jc).dma_start_transpose(
                    iw2T[jc][:, rt * P:(rt + 1) * P], blk))

            def qkv_one(jt, joff, rhsT, cols, sink):
                ps = ps_big.tile([P, cols], dt.float32, tag="dots", name="psqkv")
                for co, cw in _chunks(cols, 512):
                    for ic in range(n_i):
                        nc.tensor.matmul(
                            ps[:, co:co + cw],
                            iw1T[ic][:, joff + jt * P:joff + (jt + 1) * P],
                            rhsT[ic][:, co:co + cw],
                            start=(ic == 0), stop=(ic == n_i - 1))
                sink(jt, ps)

            def v_sink(jt, ps):
                vt = pstage.tile([P, n], dt.bfloat16, tag="qt2", name="vt")
                quantize(vt[:], ps[:], qv.lo, qv.hi, s_v, tpool=pstage)
                nc.sync.dma_start_transpose(iv_i[jt][:, :, :], vt[:])

            def q_sink(j, ps):
                quantize(iqT[j][:], ps[:], qq.lo, qq.hi, s_q, tpool=pstage)

            def k_sink(j, ps):
                quantize(ikT[j][:], ps[:], qk.lo, qk.hi, s_k, tpool=pstage)

            if not cfg.conditional:
                for jt in range(n_jq):
                    qkv_one(jt, 0, ixqT, nl, q_sink)
                    qkv_one(jt, inner, ixT, n, k_sink)
                    qkv_one(jt, 2 * inner, ixT, n, v_sink)
            else:
                # per j-tile: project q,k,v then run the two heads' attention
                # tiles; later tiles' projections overlap the ACT exp window
                # through the shared psum rotation.
                for jt in range(n_jq):
                    qkv_one(jt, 0, ixqT, nl, q_sink)
                    qkv_one(jt, inner, ixT, n, k_sink)
                    qkv_one(jt, 2 * inner, ixT, n, v_sink)
                    for h in (2 * jt, 2 * jt + 1):
                        for nt in range(n_nt):
                            attn_tile(h, nt, ps_big, h * n_nt + nt)

            # column sums + bias precompute (feeds only the dense path and
            # the bb2 broadcast; races with the attention tail)
            for jt in range(n_jq):
                psv = ps_big.tile([P, 1], dt.float32, tag="dots", name="psv")
                for s in range(n_mc):
                    nc.tensor.matmul(
                        psv[:, :], iv_i[jt][:, s, :], ones_b[:],
                        start=(s == 0), stop=(s == n_mc - 1))
                nc.vector.tensor_scalar_mul(pcorr[jt][:], psv[:], -OFF * beta)
            psw = ps_big.tile([1, dim], dt.float32, tag="dots", name="psw")
            for co, cw in _chunks(dim, 512):
                for jc in range(n_jq):
                    nc.tensor.matmul(
                        psw[:, co:co + cw], ones_b[:], iw2T[jc][:, co:co + cw],
                        start=(jc == 0), stop=(jc == n_jq - 1))
            nc.vector.tensor_scalar_mul(crow[:], psw[:], -OFF * gamma)
            nc.vector.tensor_tensor(crow[:], crow[:], bb2row[:], ALU.add)
            psb = ps_big.tile([P, dim], dt.float32, tag="dots", name="psb")
            for co, cw in _chunks(dim, 512):
                nc.tensor.matmul(psb[:, co:co + cw], ones_row[:],
                                 crow[:, co:co + cw], start=True, stop=True)
            nc.vector.tensor_copy(cbcast[:], psb[:])
            psb2 = ps_big.tile([P, dim], dt.float32, tag="dots", name="psb2")
            for co, cw in _chunks(dim, 512):
                nc.tensor.matmul(psb2[:, co:co + cw], ones_row[:],
                                 bb2row[:, co:co + cw], start=True, stop=True)
            nc.vector.tensor_copy(bcast[:], psb2[:])

        # ============ phase B: flag + branch (dense path or bb2) ===========
        with tc.tile_pool(name="attp", bufs=1) as attp, \
                tc.tile_pool(name="sstage", bufs=2) as sstage2, \
                tc.tile_pool(name="fpool", bufs=2) as fpool:
            iaT_h = attp.tile([P, n_mc, nl], dt.bfloat16, tag="iaTh")
            ix2T = [attp.tile([P, nl], dt.bfloat16, tag=f"ix2T{j}", name=f"ix2T{j}") for j in range(n_jq)]

            def attn_out(h, ps_pool):
                jt = h // 2
                po = 64 * (h % 2)
                pso = ps_pool.tile([64, nl], dt.float32, tag="attout")
                for s in range(n_mc):
                    for t8 in range(n_nt):
                        nc.tensor.matmul(
                            pso[:, t8 * P:(t8 + 1) * P],
                            iv_i[jt][:, s, po:po + 64],
                            iaT_h[:, s, t8 * P:(t8 + 1) * P],
                            start=(s == 0), stop=(s == n_mc - 1))
                quantize(ix2T[jt][po:po + 64, :], pso[:], qx2.lo, qx2.hi,
                         beta, padj=pcorr[jt][po:po + 64, :], keep_offset=True,
                         tpool=sstage2)

            def final_proj(ps_pool):
                for nt in range(n_nt):
                    psf = ps_pool.tile([P, dim], dt.float32, tag="dots")
                    for co, cw in _chunks(dim, 512):
                        for jc in range(n_jq):
                            nc.tensor.matmul(
                                psf[:, co:co + cw],
                                ix2T[jc][:, nt * P:(nt + 1) * P],
                                iw2T[jc][:, co:co + cw],
                                start=(jc == 0), stop=(jc == n_jq - 1))
                    fo = fpool.tile([P, dim], dt.float32, tag="fo")
                    nc.vector.scalar_tensor_tensor(
                        fo[:], psf[:], gamma, cbcast[:], ALU.mult, ALU.add)
                    nc.sync.dma_start(out_d[nt * P:(nt + 1) * P, :], fo[:])

            if cfg.conditional:
                emit_flag()
                flagv = nc.values_load(flag_sb.bitcast(dt.int32)[0:1, 0:1])
                with tc.tile_pool(name="ps_else", bufs=1, space="PSUM") as ps_e:
                    with tc.If(flagv < 0x3F000000) as cmp:
                        for nt in range(n_nt):
                            nc.sync.dma_start(
                                out_d[nt * P:(nt + 1) * P, :], bcast[:])
                    with cmp.Else():
                        for h in range(heads):
                            for nt in range(n_nt):
                                ia = attn_tile(h, nt, ps_e, None)
                                nc.sync.dma_start_transpose(
                                    iaT_h[:, :, nt * P:(nt + 1) * P], ia[:])
                            attn_out(h, ps_e)
                        final_proj(ps_e)
            else:
                with tc.tile_pool(name="ps_hot", bufs=1, space="PSUM") as ps_hot:
                    for h in range(heads):
                        for nt in range(n_nt):
                            ia = attn_tile(h, nt, ps_hot, h * n_nt + nt,
                                           need_ia=True)
                            nc.sync.dma_start_transpose(
                                iaT_h[:, :, nt * P:(nt + 1) * P], ia[:])
                        attn_out(h, ps_hot)
                emit_flag()
                with tc.tile_pool(name="ps_fin", bufs=2, space="PSUM") as ps_fin:
                    final_proj(ps_fin)

    nc.compile()
    return nc


# ======================== host-side entry point ===========================

_BUILD_CACHE = {}

SCALE_NAMES = ("sx1", "zx1", "sw1", "zw1", "sq", "zq", "sk", "zk",
               "sa", "za", "sv", "zv", "sx2", "zx2", "sw2", "zw2")

CONDITIONAL = os.environ.get("ATTN_KERNEL_UNCONDITIONAL", "0") != "1"
TRACE = os.environ.get("ATTN_KERNEL_TRACE", "0") == "1"

LAST_RESULTS = {}


def get_nc(cfg: Cfg):
    k = cfg.key()
    if k not in _BUILD_CACHE:
        _BUILD_CACHE[k] = build(cfg)
    return _BUILD_CACHE[k]


def make_in_maps(inputs, ncores=8):
    x = np.asarray(inputs["x"], np.float32)
    ww1 = np.ascontiguousarray(np.asarray(inputs["ww1"], np.float32))
    ww2 = np.ascontiguousarray(np.asarray(inputs["ww2"], np.float32))
    bb2 = np.ascontiguousarray(
        np.asarray(inputs["bb2"], np.float32)).reshape(1, -1)
    B, N, dim = x.shape
    halves = ncores // B
    NL = N // halves
    in_maps = []
    for c in range(ncores):
        b, hf = divmod(c, halves)
        in_maps.append({
            "xb": np.ascontiguousarray(x[b]),
            "xq": np.ascontiguousarray(x[b, hf * NL:(hf + 1) * NL]),
            "ww1": ww1, "ww2": ww2, "bb2": bb2,
        })
    return in_maps, B, N, dim, halves, NL


def kernel(**inputs) -> np.ndarray:
    scales = {k: float(np.float32(inputs[k])) for k in SCALE_NAMES}
    in_maps, B, N, dim, halves, NL = make_in_maps(inputs)
    inner = np.asarray(inputs["ww1"]).shape[0] // 3
    cfg = Cfg(dim=dim, heads=inner // 64, dh=64, n=N, nl=NL, scales=scales,
              conditional=CONDITIONAL, num_devices=8)
    nc = get_nc(cfg)
    res = bass_utils.run_bass_kernel_spmd(
        nc, in_maps, list(range(8)), trace=TRACE)
    LAST_RESULTS["res"] = res
    out = np.empty((B, N, dim), np.float32)
    for c in range(8):
        b, hf = divmod(c, halves)
        out[b, hf * NL:(hf + 1) * NL] = res.results[c]["out"]
    return out


if __name__ == "__main__":
    scales = dict(sx1=.27, zx1=8., sw1=.0107, zw1=8., sq=.15, zq=8., sk=.15,
                  zk=8., sa=1 / 15, za=0., sv=.15, zv=8., sx2=.05, zx2=8.,
                  sw2=.0107, zw2=8.)
    cfg = Cfg(dim=128, heads=2, dh=64, n=256, nl=128, scales=scales,
              conditional=True, num_devices=1)
    nc = build(cfg)
    print("build OK")

